# revision 2
# baseline (speedup 1.0000x reference)
"""Trainium2 Bass kernel for nn_DeltaResidualExpanded.

Computes, per (b, t) position:
    k    = l2normalize(sublayer_output) / sqrt(D)
    beta = 2*sigmoid(RMSNorm(x_in) @ gate_w.T + gate_b)
    v    = x_in @ Wv.T
    out  = X + beta * k (outer) (v - k.X)

Pure data-parallel over B*T across 8 NeuronCores; each core streams its
1024 positions as 8 tiles of 128 through SBUF.  The kernel is
memory-bound; the update term is a ~1e-3 relative perturbation of X, so
X / sublayer_output / x_in / OUT are carried in fp16 (host casts), which
halves HBM traffic to ~36 MB/core (~100us at 358 GB/s/core).  fp16
quantization of X and OUT contributes ~5e-4 relative error vs the 2e-2
gate.  All small per-position math stays f32.

Per 128-position tile:
  - PE: transpose x_in 128x128 blocks (fp16 PSUM), then matmul against
    [Wv.T | gate_w*gate_norm_w] accumulated in f32 PSUM
  - ACT: norms via Square-accumulate, sqrt/sigmoid, PSUM->SBUF copies
  - DVE: kTX via 8 strided scalar_tensor_tensor accumulates, the
    in-place update X[:, :, j] += sub * corr2[:, j] as 8 more, plus
    small per-position algebra

The walrus build in this container accepts at most ONE on_wait condition
per instruction, so the Tile-scheduled BIR is post-processed to hoist
extra waits into standalone EventSemaphore instructions (legalize_bir).
"""
import sys
import math

sys.path.insert(0, "/opt/trn_rl_repo")

import numpy as np

B, T, D, DV = 4, 2048, 1024, 8
N_CORES = 8
BT = B * T
CORE_BT = BT // N_CORES          # 1024 positions per core
P = 128                          # partitions per tile
NT = CORE_BT // P                # 8 tiles per core
NC_D = D // P                    # 8 d-chunks of 128
HD = D // 2
EPS_K = 1e-6
EPS_NORM = 1e-6
W_COLS = DV + 2                  # Wv rows, gate row, zero pad

_NC_CACHE: dict = {}


def legalize_bir_dict(d):
    """Split multi-wait instructions (this walrus accepts one on_wait per
    instruction): hoist extras into standalone EventSemaphore instrs."""
    n = 0
    for fn in d.get("functions", []):
        for blk in fn.get("blocks", []):
            insts = blk.get("instructions")
            if not insts:
                continue
            out = []
            for inst in insts:
                si = inst.get("sync_info")
                waits = (si or {}).get("on_wait") or []
                if len(waits) > 1:
                    for w in waits[:-1]:
                        n += 1
                        out.append({
                            "debug": inst.get("debug", 0),
                            "engine": inst["engine"],
                            "ins": [],
                            "name": f"legwait-{n}",
                            "opcode": "EventSemaphore",
                            "outs": [],
                            "sync_info": {"on_update": [], "on_wait": [w]},
                        })
                    si["on_wait"] = waits[-1:]
                out.append(inst)
            blk["instructions"] = out
    return d


def _build(gate_b_val: float, reps: int = 1, opts: dict | None = None):
    opts = dict(opts or {})
    ktx_mode = opts.get("ktx", "sttacc")         # "gpmult" | "sttacc"
    sig_func = opts.get("sig_func", "Sigmoid")   # timing-only override
    xbufs = opts.get("xbufs", 3)

    import orjson
    import concourse.bass as bass
    import concourse.tile as tile
    from concourse import mybir, masks
    from concourse.bass import ts
    from contextlib import ExitStack

    f32 = mybir.dt.float32
    f16 = mybir.dt.float16
    AF = mybir.ActivationFunctionType
    OP = mybir.AluOpType

    nc = bass.Bass()
    X = nc.dram_tensor("X", [CORE_BT, D, DV], f16, kind="ExternalInput")
    # SX = [sublayer_output | x_in] fused along the feature axis
    SX = nc.dram_tensor("SX", [CORE_BT, 2 * D], f16, kind="ExternalInput")
    # [D, W_COLS]: cols 0..7 = Wv.T, col 8 = gate_norm_w*gate_w, col 9 = 0
    WT = nc.dram_tensor("WT", [D, W_COLS], f16, kind="ExternalInput")
    OUT = nc.dram_tensor("OUT", [CORE_BT, D, DV], f16, kind="ExternalOutput")

    with tile.TileContext(nc) as tc, ExitStack() as ctx:
        consts = ctx.enter_context(tc.tile_pool(name="consts", bufs=1))
        xpool = ctx.enter_context(tc.tile_pool(name="xpool", bufs=xbufs))
        sxp = ctx.enter_context(tc.tile_pool(name="sxp", bufs=3))
        tmpp = ctx.enter_context(tc.tile_pool(name="tmpp", bufs=3))
        scrp = ctx.enter_context(tc.tile_pool(name="scrp", bufs=3))
        xtp = ctx.enter_context(tc.tile_pool(name="xtp", bufs=2))
        small = ctx.enter_context(tc.tile_pool(name="small", bufs=3))
        tpsum = ctx.enter_context(tc.tile_pool(name="tpsum", bufs=3,
                                               space="PSUM"))
        vpsum = ctx.enter_context(tc.tile_pool(name="vpsum", bufs=2,
                                               space="PSUM"))
        wpsum = ctx.enter_context(tc.tile_pool(name="wpsum", bufs=2,
                                               space="PSUM"))

        eps_sb = consts.tile([P, 1], f32)
        nc.vector.memset(eps_sb, EPS_NORM)
        ident = consts.tile([P, P], f16)
        masks.make_identity(nc, ident[:])
        # WT load as [128 d-in-chunk, chunk, col]
        wt_sb = consts.tile([P, NC_D, W_COLS], f16)
        nc.gpsimd.dma_start(
            out=wt_sb, in_=WT[:].rearrange("(c p) m -> p c m", p=P))
        # shared throwaway output for ACT accumulate ops (same-engine WAW)
        scr_act = consts.tile([P, D], f16)

        for t in range(NT * reps):
            t = t % NT
            rows = slice(t * P, (t + 1) * P)

            x_t = xpool.tile([P, D, DV], f16)
            nc.sync.dma_start(out=x_t, in_=X[rows])
            sx_t = sxp.tile([P, 2 * D], f16)
            nc.sync.dma_start(out=sx_t, in_=SX[rows])
            sub_t = sx_t[:, 0:D]
            xin_t = sx_t[:, D:2 * D]

            # ---- v & gate dot via PE: xin^T chunks, matmul with WT
            xt_sb = xtp.tile([P, D], f16)
            for c in range(NC_D):
                ps = tpsum.tile([P, P], f16, tag="tp")
                nc.tensor.transpose(ps[:], xin_t[:, ts(c, P)], ident[:])
                nc.scalar.copy(out=xt_sb[:, ts(c, P)], in_=ps[:])
            vg_ps = vpsum.tile([W_COLS, P], f32, tag="vg")
            for c in range(NC_D):
                nc.tensor.matmul(vg_ps[:, :], wt_sb[:, c, :],
                                 xt_sb[:, ts(c, P)],
                                 start=(c == 0), stop=(c == NC_D - 1))
            vg_sb = small.tile([W_COLS, P], f16, tag="vgsb")
            nc.scalar.copy(out=vg_sb[:], in_=vg_ps[:])
            vgt_ps = wpsum.tile([P, W_COLS], f16, tag="vgt")
            nc.tensor.transpose(vgt_ps[:], vg_sb[:],
                                ident[:W_COLS, :W_COLS])
            vgt = small.tile([P, W_COLS], f32, tag="vgt_sb")
            nc.scalar.copy(out=vgt[:], in_=vgt_ps[:])

            # ---- norms via ACT square-accumulate (throwaway out)
            ssq = small.tile([P, 1], f32)
            nc.scalar.activation(out=scr_act, in_=sub_t, func=AF.Square,
                                 accum_out=ssq)
            xsq = small.tile([P, 1], f32)
            nc.scalar.activation(out=scr_act, in_=xin_t, func=AF.Square,
                                 accum_out=xsq)

            # sinv = 1 / max(||sub||, EPS_K)
            snorm = small.tile([P, 1], f32)
            nc.scalar.activation(out=snorm, in_=ssq, func=AF.Sqrt)
            nc.vector.tensor_scalar_max(out=snorm, in0=snorm, scalar1=EPS_K)
            sinv = small.tile([P, 1], f32)
            nc.vector.reciprocal(out=sinv, in_=snorm)

            # rms = 1 / sqrt(mean(xin^2) + EPS_NORM)
            rmsden = small.tile([P, 1], f32)
            nc.scalar.activation(out=rmsden, in_=xsq, func=AF.Sqrt,
                                 scale=1.0 / D, bias=eps_sb)
            rms = small.tile([P, 1], f32)
            nc.vector.reciprocal(out=rms, in_=rmsden)

            # ---- raw[:, j] = sub . X[:, :, j]
            raw = small.tile([P, DV], f32)
            if ktx_mode == "gpmult":
                # GPSIMD contiguous multiply (halves), ACT strided reduce
                rawp = small.tile([P, 2, DV], f32, tag="rawp")
                for h in range(2):
                    tmp_h = tmpp.tile([P, HD, DV], f16, tag="tmp")
                    xs = x_t[:, h * HD:(h + 1) * HD, :]
                    subs = sx_t[:, h * HD:(h + 1) * HD]
                    sub_b = bass.AP(tensor=subs.tensor, offset=subs.offset,
                                    ap=[*subs.ap, [0, DV]])
                    nc.gpsimd.tensor_tensor(out=tmp_h, in0=xs, in1=sub_b,
                                            op=OP.mult)
                    for j in range(DV):
                        nc.scalar.activation(
                            out=scr_act[:, 0:HD], in_=tmp_h[:, :, j],
                            func=AF.Copy,
                            accum_out=rawp[:, h, j:j + 1])
                nc.vector.tensor_tensor(out=raw, in0=rawp[:, 0, :],
                                        in1=rawp[:, 1, :], op=OP.add)
            else:
                # fused multiply+reduce on DVE, one op per j
                for j in range(DV):
                    kx = scrp.tile([P, D], f16, tag="kx")
                    nc.vector.scalar_tensor_tensor(
                        out=kx, in0=x_t[:, :, j], scalar=1.0, in1=sub_t,
                        op0=OP.mult, op1=OP.mult,
                        accum_out=raw[:, j:j + 1])

            # ---- logit = g*rms + gate_b ; sig = sigmoid(logit)
            logit = small.tile([P, 1], f32)
            nc.vector.tensor_scalar(out=logit, in0=vgt[:, DV:DV + 1],
                                    scalar1=rms, scalar2=gate_b_val,
                                    op0=OP.mult, op1=OP.add)
            sig = small.tile([P, 1], f32)
            nc.scalar.activation(out=sig, in_=logit,
                                 func=getattr(AF, sig_func))

            # bs = 2*sig*sinv/sqrt(D);  ktx = raw*sinv/sqrt(D)
            bs = small.tile([P, 1], f32)
            nc.vector.tensor_scalar(out=bs, in0=sig, scalar1=sinv,
                                    scalar2=2.0 / math.sqrt(D),
                                    op0=OP.mult, op1=OP.mult)
            ktx = small.tile([P, DV], f32)
            nc.vector.tensor_scalar(out=ktx, in0=raw, scalar1=sinv,
                                    scalar2=1.0 / math.sqrt(D),
                                    op0=OP.mult, op1=OP.mult)
            corr = small.tile([P, DV], f32)
            nc.vector.tensor_tensor(out=corr, in0=vgt[:, 0:DV], in1=ktx,
                                    op=OP.subtract)
            corr2 = small.tile([P, DV], f32)
            nc.vector.tensor_scalar_mul(out=corr2, in0=corr, scalar1=bs)

            # ---- X[:, :, j] += sub * corr2[:, j]   (in place, DVE)
            for j in range(DV):
                nc.vector.scalar_tensor_tensor(
                    out=x_t[:, :, j], in0=sub_t, scalar=corr2[:, j:j + 1],
                    in1=x_t[:, :, j], op0=OP.mult, op1=OP.add)

            nc.scalar.dma_start(out=OUT[rows], in_=x_t)

    legal = orjson.dumps(legalize_bir_dict(nc.to_json()))
    nc.to_json_bytes = lambda: legal  # consumed by bass2jax custom-call
    return nc


def get_nc(gate_b_val: float, reps: int = 1, opts: dict | None = None):
    key = (float(gate_b_val), reps, tuple(sorted((opts or {}).items())))
    if key not in _NC_CACHE:
        _NC_CACHE[key] = _build(gate_b_val, reps, opts)
    return _NC_CACHE[key]


def make_in_maps(X, sublayer_output, x_in, gate_norm_w, gate_w, Wv):
    Xf = np.ascontiguousarray(
        np.asarray(X, dtype=np.float32).reshape(BT, D, DV)).astype(np.float16)
    SXf = np.concatenate(
        [np.asarray(sublayer_output, dtype=np.float32).reshape(BT, D),
         np.asarray(x_in, dtype=np.float32).reshape(BT, D)],
        axis=1).astype(np.float16)
    gw = (np.asarray(gate_w, dtype=np.float32).reshape(D)
          * np.asarray(gate_norm_w, dtype=np.float32).reshape(D))
    WTv = np.zeros((D, W_COLS), dtype=np.float32)
    WTv[:, :DV] = np.asarray(Wv, dtype=np.float32).T
    WTv[:, DV] = gw
    WTv = WTv.astype(np.float16)
    in_maps = []
    for c in range(N_CORES):
        sl = slice(c * CORE_BT, (c + 1) * CORE_BT)
        in_maps.append({"X": Xf[sl], "SX": SXf[sl], "WT": WTv})
    return in_maps


def kernel(X, sublayer_output, x_in, gate_norm_w, gate_w, gate_b, Wv):
    from concourse.bass_utils import run_bass_kernel_spmd

    gate_b_val = float(np.asarray(gate_b).reshape(-1)[0])
    nc = get_nc(gate_b_val)
    in_maps = make_in_maps(X, sublayer_output, x_in, gate_norm_w, gate_w, Wv)
    res = run_bass_kernel_spmd(nc, in_maps, list(range(N_CORES)))
    out = np.concatenate([res.results[c]["OUT"] for c in range(N_CORES)],
                         axis=0)
    return out.reshape(B, T, D, DV).astype(np.float32)
